# revision 1
# baseline (speedup 1.0000x reference)
"""GCNConv (rank-1 normalized aggregation) Trainium2 kernel, SPMD over 8 cores.

Math (faithful to the torch/jax reference):
    h    = x @ W
    adj  = symmetric 0/1 adjacency from edge_index (duplicates collapse: SET, not add)
    deg  = adj.sum(1);  dinv = 1/sqrt(deg)
    agg  = dinv @ h                      # rank-1 identity, [F_OUT]
    out  = dinv[:, None] * agg[None, :] + bias

Since agg = (dinv @ x) @ W, h is never materialized:
    v    = dinv @ x            ([F_IN] weighted row-sum, DVE mul + strided reduce)
    agg  = v @ W               (TensorE)
    out_c = dinv_c (x) agg + bias     (rows sharded across cores)

Collectives in this environment have a ~55us fixed latency (measured with a
bare 512B AllReduce), far above the 8-core floor, so instead of sharding the
v-reduction + AllReduce, every core reads the full x (6.1MB, ~17us at HBM BW)
and computes v locally; only the O(N*F_OUT) output is sharded.

The exact deduplicated degree (an integer/sorting problem, not a flops
problem) is computed on host with np.unique; all O(N*F) floating-point work
runs on the NeuronCores.
"""

import numpy as np

N, F_IN, F_OUT = 12000, 128, 256
N_CORES = 8
ROWS = N // N_CORES            # 1500 output rows per core
NT_OUT = 12                    # 12 row tiles per core (padded)
ROWS_PAD = NT_OUT * 128        # 1536
NT_FULL = 96                   # full-x row tiles (padded)
N_PAD = NT_FULL * 128          # 12288
# x rows-per-partition per DMA/compute chunk; small first chunks so DVE
# starts sooner, ramping up once the pipeline is primed
CHUNK_SIZES = [8, 8, 12, 12, 16, 16, 12, 12]
N_CHUNKS = len(CHUNK_SIZES)

_cache = {}


def _build_nc():
    import concourse.bacc as bacc
    import concourse.mybir as mybir
    import concourse.tile as tile

    f32 = mybir.dt.float32
    bf16 = mybir.dt.bfloat16

    nc = bacc.Bacc(
        "TRN2",
        target_bir_lowering=False,
        debug=False,
        num_devices=N_CORES,
    )

    # x and dinvT travel as bf16: halves DMA bytes and DVE mul time; the
    # ~0.3% relative error on v is far inside the 2e-2 gate
    x_d = nc.dram_tensor("x", [N_PAD, F_IN], bf16, kind="ExternalInput")
    # dinvT[p, r] = dinv[p*96 + r] (host-prepared layout matching x view)
    dinvT_d = nc.dram_tensor("dinvT", [128, NT_FULL], bf16, kind="ExternalInput")
    # f32 copy for the ScalarE activation scale operand
    dinvTf_d = nc.dram_tensor("dinvTf", [128, NT_FULL], f32, kind="ExternalInput")
    dinvS_d = nc.dram_tensor("dinvS", [128, NT_OUT], f32, kind="ExternalInput")
    w_d = nc.dram_tensor("weight", [F_IN, F_OUT], bf16, kind="ExternalInput")
    b_d = nc.dram_tensor("bias", [F_OUT], f32, kind="ExternalInput")
    out_d = nc.dram_tensor("out", [ROWS_PAD, F_OUT], f32, kind="ExternalOutput")

    # x view: partition p holds rows [p*96, (p+1)*96) -> one contiguous 48KB
    # read per partition (vs 2048 scattered 512B runs for the (n p) m view)
    x_prm = x_d.ap().rearrange("(p r) m -> p r m", p=128)      # [128,96,128]
    out_pnm = out_d.ap().rearrange("(n p) m -> p n m", p=128)  # [128,12,256]

    dma_engines = [nc.sync, nc.scalar]

    with tile.TileContext(nc) as tc:
        with (
            tc.tile_pool(name="const", bufs=1) as cpool,
            tc.tile_pool(name="xbuf", bufs=1) as xpool,
            tc.tile_pool(name="scl", bufs=3) as spool,
            tc.tile_pool(name="obuf", bufs=1) as opool,
            tc.tile_pool(name="ps", bufs=1, space="PSUM") as psum,
        ):
            # ---- small constants first (cheap), then x chunks ----
            # (keep everything off gpsimd: SWDGE completion latency is ~9us
            # and its drain blocks dependents)
            dinvT = cpool.tile([128, NT_FULL], bf16)
            nc.sync.dma_start(dinvT[:], dinvT_d.ap())
            dinvTf = cpool.tile([128, NT_FULL], f32)
            nc.scalar.dma_start(dinvTf[:], dinvTf_d.ap())
            bias_s = cpool.tile([1, F_OUT], f32)
            nc.scalar.dma_start(bias_s[:], b_d.ap().rearrange("(a n) -> a n", a=1))

            xc = []
            off = 0
            offs = []
            for q in range(N_CHUNKS):
                sz = CHUNK_SIZES[q]
                t = xpool.tile([128, sz, F_IN], bf16, tag=f"xc{q}", name=f"xc{q}")
                dma_engines[q % len(dma_engines)].dma_start(
                    t[:], x_prm[:, off : off + sz, :]
                )
                xc.append(t)
                offs.append(off)
                off += sz

            # needed only mid/late kernel; queue after the x chunks
            dinvS = cpool.tile([128, NT_OUT], f32)
            nc.scalar.dma_start(dinvS[:], dinvS_d.ap())
            w_s = cpool.tile([F_IN, F_OUT], bf16)
            nc.sync.dma_start(w_s[:], w_d.ap())

            ones_col = cpool.tile([128, 1], bf16)
            nc.vector.memset(ones_col[:], 1.0)
            ones_row = cpool.tile([1, 128], f32)
            nc.vector.memset(ones_row[:], 1.0)

            # ---- v = dinv @ x ----
            # per chunk: scaled = x * dinv (DVE); TensorE contracts partitions
            # via ones-matmuls, ALL accumulating into one [1,512] PSUM bank:
            # pvw[0, u] = sum over rows r with r%4 == u//128 of dinv_r*x[r, u%128]
            pvw = psum.tile([1, 512], f32)
            total_sl = sum(CHUNK_SIZES) * F_IN // 512
            sl = 0
            for q in range(N_CHUNKS):
                sz = CHUNK_SIZES[q]
                d_bc = (
                    dinvT[:, offs[q] : offs[q] + sz]
                    .unsqueeze(2)
                    .broadcast_to([128, sz, F_IN])
                )
                scaled = spool.tile([128, sz, F_IN], bf16, tag=f"scaled{q % 3}",
                                    name=f"scaled{q}")
                if q >= N_CHUNKS - 6:
                    # late chunks: split the scaling DVE/ScalarE so the
                    # pipeline tail shortens (ACT does the last 4 rows;
                    # by then the Activation sequencer has issued all DMAs)
                    dv = sz - 4
                    nc.vector.tensor_mul(
                        scaled[:, :dv, :], xc[q][:, :dv, :],
                        d_bc[:, :dv, :],
                    )
                    for r in range(dv, sz):
                        nc.scalar.activation(
                            scaled[:, r, :],
                            xc[q][:, r, :],
                            mybir.ActivationFunctionType.Copy,
                            scale=dinvTf[:, offs[q] + r : offs[q] + r + 1],
                        )
                else:
                    nc.vector.tensor_mul(scaled[:], xc[q][:], d_bc)
                flat = scaled[:].rearrange("p t j -> p (t j)")
                for s in range((sz * F_IN) // 512):
                    nc.tensor.matmul(
                        pvw[:],
                        ones_col[:],
                        flat[:, s * 512 : (s + 1) * 512],
                        start=(sl == 0),
                        stop=(sl == total_sl - 1),
                        skip_group_check=True,
                    )
                    sl += 1
            # fold the 4 t-mod groups: one small strided reduce
            vrow = cpool.tile([1, F_IN], f32)
            nc.vector.tensor_reduce(
                vrow[:],
                pvw[:].rearrange("a (t j) -> a j t", j=F_IN),
                axis=mybir.AxisListType.X,
                op=mybir.AluOpType.add,
            )

            # v [1,128] -> vcol [128,1] via TensorE transpose; cast to bf16
            # (for the A2 matmul whose rhs W is bf16) in the PSUM->SBUF copy
            pvcol = psum.tile([F_IN, 1], f32)
            nc.tensor.transpose(pvcol[:], vrow[:], ones_row[:1, :1])
            vcol = cpool.tile([F_IN, 1], bf16)
            nc.vector.tensor_copy(vcol[:], pvcol[:])

            # ---- A2[p, o] = agg[o] = sum_j v[j] W[j, o]  (v bcast as lhsT) ----
            pA2 = psum.tile([128, F_OUT], f32)
            nc.tensor.matmul(
                pA2[:],
                vcol[:].broadcast_to([F_IN, 128]),
                w_s[:],
                start=True,
                stop=True,
            )
            A2 = cpool.tile([128, F_OUT], f32)
            nc.vector.tensor_copy(A2[:], pA2[:])
            pB2 = psum.tile([128, F_OUT], f32)
            nc.tensor.matmul(pB2[:], ones_row[:], bias_s[:], start=True, stop=True)
            B2 = cpool.tile([128, F_OUT], f32)
            nc.vector.tensor_copy(B2[:], pB2[:])

            # ---- out tile i = (A2 * dinvS_i) + B2, one fused DVE op each ----
            # shrinking DMA groups so the last transfer is small
            out_engines = [nc.sync, nc.scalar]
            og_sizes = [3, 3, 2, 2, 1, 1]
            base = 0
            for g, gsz in enumerate(og_sizes):
                og = opool.tile([128, gsz, F_OUT], f32, tag=f"og{g}",
                                name=f"og{g}")
                for j in range(gsz):
                    i = base + j
                    nc.vector.scalar_tensor_tensor(
                        og[:, j, :],
                        A2[:],
                        dinvS[:, i : i + 1],
                        B2[:],
                        op0=mybir.AluOpType.mult,
                        op1=mybir.AluOpType.add,
                    )
                out_engines[g % 2].dma_start(
                    out_pnm[:, base : base + gsz, :], og[:]
                )
                base += gsz

    nc.compile()
    return nc


def _get_nc():
    if "nc" not in _cache:
        _cache["nc"] = _build_nc()
    return _cache["nc"]


def _host_dinv(edge_index: np.ndarray) -> np.ndarray:
    """Exact deduplicated symmetric degree -> 1/sqrt(deg), matching
    adj[a,b]=1; adj[b,a]=1; deg=adj.sum(1)."""
    a = edge_index[0].astype(np.int64)
    b = edge_index[1].astype(np.int64)
    keys = np.unique(np.concatenate([a * N + b, b * N + a]))
    deg = np.bincount(keys // N, minlength=N).astype(np.float32)
    with np.errstate(divide="ignore"):
        dinv = (np.float32(1.0) / np.sqrt(deg)).astype(np.float32)
    return dinv


def kernel(x, edge_index, weight, bias, _trace=False):
    from concourse import bass_utils

    x = np.ascontiguousarray(x, dtype=np.float32)
    weight = np.ascontiguousarray(weight, dtype=np.float32)
    bias = np.ascontiguousarray(bias, dtype=np.float32)
    dinv = _host_dinv(np.asarray(edge_index))

    nc = _get_nc()

    import ml_dtypes

    bf16 = ml_dtypes.bfloat16
    xp = np.zeros((N_PAD, F_IN), bf16)
    xp[:N] = x.astype(bf16)
    dp = np.zeros((N_PAD,), np.float32)
    dp[:N] = dinv
    # dinvT[p, r] = dinv[p*96 + r], matching the x view "(p r) m -> p r m"
    dinvTf = np.ascontiguousarray(dp.reshape(128, NT_FULL))
    dinvT = dinvTf.astype(bf16)

    w16 = weight.astype(bf16)
    in_maps = []
    for c in range(N_CORES):
        r0 = c * ROWS
        ds = np.zeros((ROWS_PAD,), np.float32)
        ds[:ROWS] = dinv[r0 : r0 + ROWS]
        dinvS = np.ascontiguousarray(ds.reshape(NT_OUT, 128).T)  # [128, 12]
        in_maps.append(
            {
                "x": xp,
                "dinvT": dinvT,
                "dinvTf": dinvTf,
                "dinvS": dinvS,
                "weight": w16,
                "bias": bias,
            }
        )

    res = bass_utils.run_bass_kernel_spmd(
        nc, in_maps, core_ids=list(range(N_CORES)), trace=_trace
    )
    out = np.concatenate(
        [res.results[c]["out"][:ROWS] for c in range(N_CORES)], axis=0
    )
    if _trace:
        _cache["last_results"] = res
    return out



# revision 5
# speedup vs baseline: 1.0679x; 1.0679x over previous
"""GCNConv (rank-1 normalized aggregation) Trainium2 kernel, SPMD over 8 cores.

Math (faithful to the torch/jax reference):
    h    = x @ W
    adj  = symmetric 0/1 adjacency from edge_index (duplicates collapse: SET, not add)
    deg  = adj.sum(1);  dinv = 1/sqrt(deg)
    agg  = dinv @ h                      # rank-1 identity, [F_OUT]
    out  = dinv[:, None] * agg[None, :] + bias

Since agg = (dinv @ x) @ W, h is never materialized.  Per core:
    v    = dinv @ x      96 accumulating TensorE matmuls, lhsT = one dinv
                         column [128,1], rhs = one x row-slice [128,128].
                         No DVE pre-multiply; TensorE eats x straight from
                         the DMA stream.
    agg  = v @ W         one matmul (after a tiny transpose of v)
    out  = dinv_c (x) [agg; bias]   12 outer-product matmuls with a [2,128]
                         stationary ([dinv ; ones]) and [2,256] moving
                         ([agg ; bias]); results DMA'd HBM-ward directly
                         from PSUM (no SBUF bounce).

Collectives here have a ~55us fixed latency (measured), far above the
8-core floor, so every core reads the full x (3.1MB bf16, ~9us at HBM BW)
and computes v locally; only the O(N*F_OUT) output is sharded.

Output layout per core is p-major ("(p n) m"): partition p holds 12
consecutive output rows, so the store DMA has 4KB-contiguous runs.

The exact deduplicated degree (an integer/sorting problem, not a flops
problem) is computed on host with np.unique; all O(N*F) floating-point work
runs on the NeuronCores.
"""

import numpy as np

N, F_IN, F_OUT = 12000, 128, 256
N_CORES = 8
ROWS = N // N_CORES            # 1500 output rows per core
NT_OUT = 12                    # 12 row tiles per core (padded)
ROWS_PAD = NT_OUT * 128        # 1536
NT_FULL = 96                   # full-x row slices (padded)
N_PAD = NT_FULL * 128          # 12288
# x row-slices per DMA chunk; small first chunks so TensorE starts sooner
CHUNK_SIZES = [8, 8, 16, 16, 16, 16, 16]
N_CHUNKS = len(CHUNK_SIZES)
N_WARM = 14                    # dummy matmuls to keep the PE HAM window busy
OG = 2                         # out tiles per store DMA group

_cache = {}


def _build_nc():
    import concourse.bacc as bacc
    import concourse.mybir as mybir
    import concourse.tile as tile

    f32 = mybir.dt.float32
    bf16 = mybir.dt.bfloat16

    nc = bacc.Bacc(
        "TRN2",
        target_bir_lowering=False,
        debug=False,
        num_devices=N_CORES,
    )

    # x and dinvT travel as bf16: halves DMA bytes; the ~0.3% relative
    # error on v is far inside the 2e-2 gate
    x_d = nc.dram_tensor("x", [N_PAD, F_IN], bf16, kind="ExternalInput")
    # dinvT[p, r] = dinv[p*96 + r] (host-prepared layout matching x view)
    dinvT_d = nc.dram_tensor("dinvT", [128, NT_FULL], bf16, kind="ExternalInput")
    # dinvOnes[0, n*128+p] = dinv_core[p*12+n]; dinvOnes[1, :] = 1.0
    dinvOnes_d = nc.dram_tensor("dinvOnes", [2, ROWS_PAD], bf16, kind="ExternalInput")
    w_d = nc.dram_tensor("weight", [F_IN, F_OUT], bf16, kind="ExternalInput")
    b_d = nc.dram_tensor("biasbf", [1, F_OUT], bf16, kind="ExternalInput")
    out_d = nc.dram_tensor("out", [ROWS_PAD, F_OUT], f32, kind="ExternalOutput")

    # x view: partition p holds rows [p*96, (p+1)*96) -> one contiguous 24KB
    # read per partition
    x_prm = x_d.ap().rearrange("(p r) m -> p r m", p=128)      # [128,96,128]
    # out view: partition p holds rows [p*12, (p+1)*12) -> 4KB-contiguous
    # store runs per DMA group
    out_pnm = out_d.ap().rearrange("(p n) m -> p n m", p=128)  # [128,12,256]

    with tile.TileContext(nc) as tc:
        with (
            tc.tile_pool(name="const", bufs=1) as cpool,
            tc.tile_pool(name="xbuf", bufs=1) as xpool,
            tc.tile_pool(name="pvp", bufs=1, space="PSUM") as pvpool,
            tc.tile_pool(name="ptp", bufs=1, space="PSUM") as ptpool,
            tc.tile_pool(name="pot", bufs=2, space="PSUM") as potpool,
            tc.tile_pool(name="obuf", bufs=2) as opool,
        ):
            # ---- DMAs: first x chunk + dinvT lead; everything else follows
            xc = []
            offs = []
            off = 0
            dinvT = cpool.tile([128, NT_FULL], bf16)
            for q in range(N_CHUNKS):
                sz = CHUNK_SIZES[q]
                t = xpool.tile([128, sz, F_IN], bf16, tag=f"xc{q}", name=f"xc{q}")
                (nc.sync if q % 2 == 0 else nc.scalar).dma_start(
                    t[:], x_prm[:, off : off + sz, :]
                )
                if q == 0:
                    nc.scalar.dma_start(dinvT[:], dinvT_d.ap())
                xc.append(t)
                offs.append(off)
                off += sz

            # consts needed only mid/late kernel; queued after the x chunks
            w_s = cpool.tile([F_IN, F_OUT], bf16)
            nc.scalar.dma_start(w_s[:], w_d.ap())
            dinvOnes = cpool.tile([2, ROWS_PAD], bf16)
            nc.scalar.dma_start(dinvOnes[:], dinvOnes_d.ap())
            aggbias = cpool.tile([2, F_OUT], bf16)
            nc.scalar.dma_start(aggbias[1:2, :], b_d.ap())

            one11 = cpool.tile([1, 1], f32)
            nc.vector.memset(one11[:], 1.0)
            wtile = cpool.tile([128, 64], bf16)
            nc.vector.memset(wtile[:], 0.0)

            # ---- PE warmup: dummy matmuls fill the HAM activity window so
            # the PE clock is at 2.4GHz when the real stream begins
            pwarm = ptpool.tile([1, 64], f32, tag="pwarm", name="pwarm")
            for i in range(N_WARM):
                nc.tensor.matmul(
                    pwarm[:],
                    wtile[:, 0:1],
                    wtile[:],
                    start=True,
                    stop=True,
                    skip_group_check=True,
                )

            # ---- v = dinv @ x : 96 accumulating matmuls into one [1,128]
            # PSUM bank; lhsT = dinv column (stationary), rhs = x row-slice
            pv = pvpool.tile([1, F_IN], f32)
            r = 0
            for q in range(N_CHUNKS):
                sz = CHUNK_SIZES[q]
                for j in range(sz):
                    nc.tensor.matmul(
                        pv[:],
                        dinvT[:, r : r + 1],
                        xc[q][:, j, :],
                        start=(r == 0),
                        stop=(r == NT_FULL - 1),
                        skip_group_check=True,
                    )
                    r += 1

            # v [1,128] -> vcol [128,1] via TensorE transpose; cast to bf16
            vrow = cpool.tile([1, F_IN], f32)
            nc.vector.tensor_copy(vrow[:], pv[:])
            pvt = ptpool.tile([F_IN, 1], f32, tag="pvt", name="pvt")
            nc.tensor.transpose(pvt[:], vrow[:], one11[:])
            vcol = cpool.tile([F_IN, 1], bf16)
            nc.vector.tensor_copy(vcol[:], pvt[:])

            # agg[o] = sum_j v[j] W[j, o]  -> aggbias row 0 (bf16)
            pagg = ptpool.tile([1, F_OUT], f32, tag="pagg", name="pagg")
            nc.tensor.matmul(pagg[:], vcol[:], w_s[:], start=True, stop=True)
            nc.vector.tensor_copy(aggbias[0:1, :], pagg[:])

            # ---- out tile n = outer(dinv_n, agg) + outer(1, bias), one
            # contraction-2 matmul each; PSUM -> SBUF copies alternate
            # between VectorE and ScalarE, store DMAs between sync/scalar
            out_engines = [nc.sync, nc.scalar]
            for g in range(NT_OUT // OG):
                pot = potpool.tile([128, OG, F_OUT], f32, tag=f"pot{g % 2}",
                                   name=f"pot{g}")
                for j in range(OG):
                    n = g * OG + j
                    nc.tensor.matmul(
                        pot[:, j, :],
                        dinvOnes[:, n * 128 : (n + 1) * 128],
                        aggbias[:],
                        start=True,
                        stop=True,
                        skip_group_check=True,
                    )
                og = opool.tile([128, OG, F_OUT], f32, tag=f"og{g % 2}",
                                name=f"og{g}")
                if g % 2 == 0:
                    nc.vector.tensor_copy(og[:], pot[:])
                else:
                    nc.scalar.activation(
                        og[:], pot[:], mybir.ActivationFunctionType.Copy
                    )
                out_engines[g % 2].dma_start(
                    out_pnm[:, g * OG : (g + 1) * OG, :], og[:]
                )

    nc.compile()
    return nc


def _get_nc():
    if "nc" not in _cache:
        _cache["nc"] = _build_nc()
    return _cache["nc"]


def _host_dinv(edge_index: np.ndarray) -> np.ndarray:
    """Exact deduplicated symmetric degree -> 1/sqrt(deg), matching
    adj[a,b]=1; adj[b,a]=1; deg=adj.sum(1)."""
    a = edge_index[0].astype(np.int64)
    b = edge_index[1].astype(np.int64)
    keys = np.unique(np.concatenate([a * N + b, b * N + a]))
    deg = np.bincount(keys // N, minlength=N).astype(np.float32)
    with np.errstate(divide="ignore"):
        dinv = (np.float32(1.0) / np.sqrt(deg)).astype(np.float32)
    return dinv


def kernel(x, edge_index, weight, bias, _trace=False):
    from concourse import bass_utils

    x = np.ascontiguousarray(x, dtype=np.float32)
    weight = np.ascontiguousarray(weight, dtype=np.float32)
    bias = np.ascontiguousarray(bias, dtype=np.float32)
    dinv = _host_dinv(np.asarray(edge_index))

    nc = _get_nc()

    import ml_dtypes

    bf16 = ml_dtypes.bfloat16
    xp = np.zeros((N_PAD, F_IN), bf16)
    xp[:N] = x.astype(bf16)
    dp = np.zeros((N_PAD,), np.float32)
    dp[:N] = dinv
    # dinvT[p, r] = dinv[p*96 + r], matching the x view "(p r) m -> p r m"
    dinvT = np.ascontiguousarray(dp.reshape(128, NT_FULL)).astype(bf16)

    w16 = weight.astype(bf16)
    b16 = bias.reshape(1, F_OUT).astype(bf16)
    in_maps = []
    for c in range(N_CORES):
        r0 = c * ROWS
        ds = np.zeros((ROWS_PAD,), np.float32)
        ds[:ROWS] = dinv[r0 : r0 + ROWS]
        # out row p*12+n lives on partition p; outer-product lhsT for tile n
        # needs dinv_core[p*12+n] at position n*128+p
        do = np.ones((2, ROWS_PAD), np.float32)
        do[0] = ds.reshape(128, NT_OUT).T.reshape(-1)
        in_maps.append(
            {
                "x": xp,
                "dinvT": dinvT,
                "dinvOnes": do.astype(bf16),
                "weight": w16,
                "biasbf": b16,
            }
        )

    res = bass_utils.run_bass_kernel_spmd(
        nc, in_maps, core_ids=list(range(N_CORES)), trace=_trace
    )
    out = np.concatenate(
        [res.results[c]["out"][:ROWS] for c in range(N_CORES)], axis=0
    )
    if _trace:
        _cache["last_results"] = res
    return out


# revision 6
# speedup vs baseline: 1.1760x; 1.1012x over previous
"""GCNConv (rank-1 normalized aggregation) Trainium2 kernel, SPMD over 8 cores.

Math (faithful to the torch/jax reference):
    h    = x @ W
    adj  = symmetric 0/1 adjacency from edge_index (duplicates collapse: SET, not add)
    deg  = adj.sum(1);  dinv = 1/sqrt(deg)
    agg  = dinv @ h                      # rank-1 identity, [F_OUT]
    out  = dinv[:, None] * agg[None, :] + bias

Since agg = (dinv @ x) @ W, h is never materialized.  Per core:
    v    = dinv @ x      96 accumulating TensorE matmuls, lhsT = one dinv
                         column [128,1], rhs = one x row-slice [128,128].
                         No DVE pre-multiply; TensorE eats x straight from
                         the DMA stream.
    agg  = v @ W         one matmul (after a tiny transpose of v)
    out  = dinv_c (x) [agg; bias]   12 outer-product matmuls with a [2,128]
                         stationary ([dinv ; ones]) and [2,256] moving
                         ([agg ; bias]); results DMA'd HBM-ward directly
                         from PSUM (no SBUF bounce).

Collectives here have a ~55us fixed latency (measured), far above the
8-core floor, so every core reads the full x (3.1MB bf16, ~9us at HBM BW)
and computes v locally; only the O(N*F_OUT) output is sharded.

Output layout per core is p-major ("(p n) m"): partition p holds 12
consecutive output rows, so the store DMA has 4KB-contiguous runs.

The exact deduplicated degree (an integer/sorting problem, not a flops
problem) is computed on host with np.unique; all O(N*F) floating-point work
runs on the NeuronCores.
"""

import numpy as np

N, F_IN, F_OUT = 12000, 128, 256
N_CORES = 8
ROWS = N // N_CORES            # 1500 output rows per core
NT_OUT = 12                    # 12 row tiles per core (padded)
ROWS_PAD = NT_OUT * 128        # 1536
NT_FULL = 96                   # full-x row slices (padded)
N_PAD = NT_FULL * 128          # 12288
# x row-slices per DMA chunk; small first chunks so TensorE starts sooner
CHUNK_SIZES = [8, 16, 16, 16, 16, 16, 8]
N_CHUNKS = len(CHUNK_SIZES)
N_WARM = 60                    # dummy matmuls to keep the PE HAM window busy
OG = 2                         # out tiles per store DMA group

_cache = {}


def _build_nc():
    import concourse.bacc as bacc
    import concourse.mybir as mybir
    import concourse.tile as tile

    f32 = mybir.dt.float32
    bf16 = mybir.dt.bfloat16

    nc = bacc.Bacc(
        "TRN2",
        target_bir_lowering=False,
        debug=False,
        num_devices=N_CORES,
    )

    # x and dinvT travel as bf16: halves DMA bytes; the ~0.3% relative
    # error on v is far inside the 2e-2 gate
    x_d = nc.dram_tensor("x", [N_PAD, F_IN], bf16, kind="ExternalInput")
    # dinvT[p, r] = dinv[p*96 + r] (host-prepared layout matching x view)
    dinvT_d = nc.dram_tensor("dinvT", [128, NT_FULL], bf16, kind="ExternalInput")
    # dinvOnes[0, n*128+p] = dinv_core[p*12+n]; dinvOnes[1, :] = 1.0
    dinvOnes_d = nc.dram_tensor("dinvOnes", [2, ROWS_PAD], bf16, kind="ExternalInput")
    w_d = nc.dram_tensor("weight", [F_IN, F_OUT], bf16, kind="ExternalInput")
    b_d = nc.dram_tensor("biasbf", [1, F_OUT], bf16, kind="ExternalInput")
    out_d = nc.dram_tensor("out", [ROWS_PAD, F_OUT], bf16, kind="ExternalOutput")

    # x view: partition p holds rows [p*96, (p+1)*96) -> one contiguous 24KB
    # read per partition
    x_prm = x_d.ap().rearrange("(p r) m -> p r m", p=128)      # [128,96,128]
    # out view: partition p holds rows [p*12, (p+1)*12) -> 4KB-contiguous
    # store runs per DMA group
    out_pnm = out_d.ap().rearrange("(p n) m -> p n m", p=128)  # [128,12,256]

    with tile.TileContext(nc) as tc:
        with (
            tc.tile_pool(name="const", bufs=1) as cpool,
            tc.tile_pool(name="xbuf", bufs=1) as xpool,
            tc.tile_pool(name="pvp", bufs=1, space="PSUM") as pvpool,
            tc.tile_pool(name="ptp", bufs=1, space="PSUM") as ptpool,
            tc.tile_pool(name="pot", bufs=2, space="PSUM") as potpool,
            tc.tile_pool(name="obuf", bufs=2) as opool,
        ):
            # ---- DMAs: first x chunk + dinvT lead; everything else follows
            xc = []
            offs = []
            off = 0
            dinvT = cpool.tile([128, NT_FULL], bf16)
            for q in range(N_CHUNKS):
                sz = CHUNK_SIZES[q]
                t = xpool.tile([128, sz, F_IN], bf16, tag=f"xc{q}", name=f"xc{q}")
                if q == 0:
                    nc.scalar.dma_start(dinvT[:], dinvT_d.ap())
                nc.sync.dma_start(t[:], x_prm[:, off : off + sz, :])
                xc.append(t)
                offs.append(off)
                off += sz

            # consts needed only mid/late kernel; queued after the x chunks
            w_s = cpool.tile([F_IN, F_OUT], bf16)
            nc.scalar.dma_start(w_s[:], w_d.ap())
            dinvOnes = cpool.tile([2, ROWS_PAD], bf16)
            nc.scalar.dma_start(dinvOnes[:], dinvOnes_d.ap())
            aggbias = cpool.tile([2, F_OUT], bf16)
            nc.scalar.dma_start(aggbias[1:2, :], b_d.ap())

            one11 = cpool.tile([1, 1], f32)
            nc.vector.memset(one11[:], 1.0)
            wtile = cpool.tile([128, 4], bf16)
            nc.vector.memset(wtile[:], 0.0)

            # ---- PE warmup: dummy matmuls fill the HAM activity window so
            # the PE clock is at 2.4GHz when the real stream begins
            pwarm = ptpool.tile([1, 4], f32, tag="pwarm", name="pwarm")
            for i in range(N_WARM):
                nc.tensor.matmul(
                    pwarm[:],
                    wtile[:, 0:1],
                    wtile[:],
                    start=True,
                    stop=True,
                    skip_group_check=True,
                )

            # ---- v = dinv @ x : 96 accumulating matmuls into one [1,128]
            # PSUM bank; lhsT = dinv column (stationary), rhs = x row-slice
            pv = pvpool.tile([1, F_IN], f32)
            r = 0
            for q in range(N_CHUNKS):
                sz = CHUNK_SIZES[q]
                for j in range(sz):
                    nc.tensor.matmul(
                        pv[:],
                        dinvT[:, r : r + 1],
                        xc[q][:, j, :],
                        start=(r == 0),
                        stop=(r == NT_FULL - 1),
                        skip_group_check=True,
                    )
                    r += 1

            # v [1,128] -> vcol [128,1] via TensorE transpose; cast to bf16
            vrow = cpool.tile([1, F_IN], f32)
            nc.vector.tensor_copy(vrow[:], pv[:])
            pvt = ptpool.tile([F_IN, 1], f32, tag="pvt", name="pvt")
            nc.tensor.transpose(pvt[:], vrow[:], one11[:])
            vcol = cpool.tile([F_IN, 1], bf16)
            nc.vector.tensor_copy(vcol[:], pvt[:])

            # agg[o] = sum_j v[j] W[j, o]  -> aggbias row 0 (bf16)
            pagg = ptpool.tile([1, F_OUT], f32, tag="pagg", name="pagg")
            nc.tensor.matmul(pagg[:], vcol[:], w_s[:], start=True, stop=True)
            nc.vector.tensor_copy(aggbias[0:1, :], pagg[:])

            # ---- out tile n = outer(dinv_n, agg) + outer(1, bias), one
            # contraction-2 matmul each; PSUM -> SBUF copies alternate
            # between VectorE and ScalarE, store DMAs between sync/scalar
            out_engines = [nc.sync, nc.scalar]
            for g in range(NT_OUT // OG):
                pot = potpool.tile([128, OG, F_OUT], f32, tag=f"pot{g % 2}",
                                   name=f"pot{g}")
                for j in range(OG):
                    n = g * OG + j
                    nc.tensor.matmul(
                        pot[:, j, :],
                        dinvOnes[:, n * 128 : (n + 1) * 128],
                        aggbias[:],
                        start=True,
                        stop=True,
                        skip_group_check=True,
                    )
                og = opool.tile([128, OG, F_OUT], bf16, tag=f"og{g % 2}",
                                name=f"og{g}")
                if g % 2 == 0:
                    nc.vector.tensor_copy(og[:], pot[:])
                else:
                    nc.scalar.activation(
                        og[:], pot[:], mybir.ActivationFunctionType.Copy
                    )
                out_engines[g % 2].dma_start(
                    out_pnm[:, g * OG : (g + 1) * OG, :], og[:]
                )

    nc.compile()
    return nc


def _get_nc():
    if "nc" not in _cache:
        _cache["nc"] = _build_nc()
    return _cache["nc"]


def _host_dinv(edge_index: np.ndarray) -> np.ndarray:
    """Exact deduplicated symmetric degree -> 1/sqrt(deg), matching
    adj[a,b]=1; adj[b,a]=1; deg=adj.sum(1)."""
    a = edge_index[0].astype(np.int64)
    b = edge_index[1].astype(np.int64)
    keys = np.unique(np.concatenate([a * N + b, b * N + a]))
    deg = np.bincount(keys // N, minlength=N).astype(np.float32)
    with np.errstate(divide="ignore"):
        dinv = (np.float32(1.0) / np.sqrt(deg)).astype(np.float32)
    return dinv


def kernel(x, edge_index, weight, bias, _trace=False):
    from concourse import bass_utils

    x = np.ascontiguousarray(x, dtype=np.float32)
    weight = np.ascontiguousarray(weight, dtype=np.float32)
    bias = np.ascontiguousarray(bias, dtype=np.float32)
    dinv = _host_dinv(np.asarray(edge_index))

    nc = _get_nc()

    import ml_dtypes

    bf16 = ml_dtypes.bfloat16
    xp = np.zeros((N_PAD, F_IN), bf16)
    xp[:N] = x.astype(bf16)
    dp = np.zeros((N_PAD,), np.float32)
    dp[:N] = dinv
    # dinvT[p, r] = dinv[p*96 + r], matching the x view "(p r) m -> p r m"
    dinvT = np.ascontiguousarray(dp.reshape(128, NT_FULL)).astype(bf16)

    w16 = weight.astype(bf16)
    b16 = bias.reshape(1, F_OUT).astype(bf16)
    in_maps = []
    for c in range(N_CORES):
        r0 = c * ROWS
        ds = np.zeros((ROWS_PAD,), np.float32)
        ds[:ROWS] = dinv[r0 : r0 + ROWS]
        # out row p*12+n lives on partition p; outer-product lhsT for tile n
        # needs dinv_core[p*12+n] at position n*128+p
        do = np.ones((2, ROWS_PAD), np.float32)
        do[0] = ds.reshape(128, NT_OUT).T.reshape(-1)
        in_maps.append(
            {
                "x": xp,
                "dinvT": dinvT,
                "dinvOnes": do.astype(bf16),
                "weight": w16,
                "biasbf": b16,
            }
        )

    res = bass_utils.run_bass_kernel_spmd(
        nc, in_maps, core_ids=list(range(N_CORES)), trace=_trace
    )
    out = np.concatenate(
        [res.results[c]["out"][:ROWS].astype(np.float32) for c in range(N_CORES)],
        axis=0,
    )
    if _trace:
        _cache["last_results"] = res
    return out
